# revision 44
# baseline (speedup 1.0000x reference)
"""EVA-02 ViT attention block (LoRA + rope + rel-pos-bias) on 8 TRN2 NeuronCores.

Data-parallel over batch (8 images per core), all matmuls bf16 (1 cyc/row).
Per core:
  - LoRA merged into qkv weights on the host; q-scale and v-bias folded away.
  - q/k projected in transposed layout (channels on partitions), v natural.
  - q-bias added during PSUM eviction (Act Identity+bias); rope is pure
    bf16 tensor ops on DVE (2x mode), pair-swap via stream_shuffle.
  - scores transposed (S^T[j,i]), one matmul per (head, j-chunk). Walrus
    quadrant-tile constraint: all start=True openers within one PSUM bank
    must share the lhsT base partition, so ph=0 heads (base 0) and ph=1
    heads (base 64) accumulate in separate banks, at 1KB-aligned offsets.
  - rel-pos bias folded multiplicatively: probs *= exp(rpb) (host table),
    A-chunk on Pool, B-chunk on DVE — off the rope-saturated DVE FIFO.
  - exp on ScalarE without max subtraction (scores are O(1)); probs bf16,
    both j-chunks of a head-pair written by one strided-AP activation.
  - attn@v with v stationary emits O^T; softmax denominators via
    ones-vector matmuls; 1/x via DVE reciprocal_approx_fast (avoids the
    Ln/Exp act-table ping-pong); reciprocals broadcast to all partitions
    through a DRAM bounce (SBUF->SBUF DMA cannot replicate rows).
  - per-quad software pipeline: rope units interleaved into the first
    attention's head loop; phase1 (scores..sums) of both pairs runs before
    normalization so PE never waits on the reciprocal chain; the output
    projection runs on quad-wide O^T tiles (788 tokens -> 7 chunk-groups
    instead of 8, stores through a flattened [img*n, c] DRAM view);
    xt prefetched one pair ahead; first-use-ordered constant DMAs
    shorten the cold start.
"""
import numpy as np
import ml_dtypes

B, N, C, H, R = 64, 197, 768, 12, 24
D = C // H               # 64
NCORES = 8
BPC = B // NCORES        # images per core
F2 = 2 * N               # 394
F4 = 4 * N               # 788
N0, N1 = 128, N - 128    # token chunks: 128 + 69

_cache = {}

SHUF_MASK = list(range(16, 32)) + list(range(0, 16))
ROPE_ORDER = [0, 6, 1, 7, 2, 8, 3, 9, 4, 10, 5, 11]


def _perm64():
    p = []
    for blk in range(2):
        base = blk * 32
        p += [base + 2 * t for t in range(16)]
        p += [base + 2 * t + 1 for t in range(16)]
    return np.array(p)


def build_program(n_pairs=BPC // 2, use_shuffle=True, repeat=1):
    import concourse.bass as bass
    import concourse.tile as tile
    from concourse import bacc, mybir

    f32, f32r, bf16 = mybir.dt.float32, mybir.dt.float32r, mybir.dt.bfloat16
    AF = mybir.ActivationFunctionType
    OP = mybir.AluOpType

    nc = bacc.Bacc("TRN2", target_bir_lowering=False, debug=False)
    n_img = 2 * n_pairs

    xt_d = nc.dram_tensor("xt", [n_pairs, C, F2], bf16, kind="ExternalInput")
    wt_d = nc.dram_tensor("wt", [C, 3 * C], bf16, kind="ExternalInput")
    bq_d = nc.dram_tensor("bq", [128, 6], f32, kind="ExternalInput")
    cs_d = nc.dram_tensor("cs", [2, 128, F4], bf16, kind="ExternalInput")
    erpe_d = nc.dram_tensor("erpe", [6, 2, 128, F2], bf16, kind="ExternalInput")
    projt_d = nc.dram_tensor("projt", [C, C], bf16, kind="ExternalInput")
    projb_d = nc.dram_tensor("projb", [1, C], f32, kind="ExternalInput")
    y_d = nc.dram_tensor("y", [n_img, N, C], f32, kind="ExternalOutput")
    # DRAM bounce buffer for the softmax-reciprocal broadcast (SBUF->SBUF
    # DMAs cannot replicate one partition row to many)
    rsf_d = nc.dram_tensor("rsf_scratch", [2, 2, 6 * F2], f32, kind="Internal")

    from contextlib import ExitStack
    with tile.TileContext(nc) as tc:
        with ExitStack() as stk:
            pool = lambda name, bufs, **kw: stk.enter_context(
                tc.tile_pool(name=name, bufs=bufs, **kw))
            # NOTE: bufs is per-tag. PSUM budget: qkps 2 + yps 2 (shared by
            # v-proj and y-proj) + ps0 1 + ps1 1 + aops 1 + sums 1 = 8 banks.
            constp = pool("const", 1)
            xtp = pool("xt", 2)
            qkps = pool("qkps", 2, space="PSUM")
            qkbfp = pool("qkbf", 2)
            ropet = pool("ropet", 1)
            vsbp = pool("vsb", 8)
            scps = pool("scps", 1, space="PSUM")
            probsp = pool("probs", 4)
            aops = pool("aops", 1, space="PSUM")
            sumsp = pool("sums", 1, space="PSUM")
            rsbp = pool("rsb", 2)
            aosbp = pool("aosb", 12)
            rbc = pool("rbc", 1)
            yps = pool("yps", 2, space="PSUM")
            ysbp = pool("ysb", 2)
            otp = pool("otp", 6)

            total_pairs = repeat * n_pairs
            xt_pre = {}

            def load_xt(pi):
                if pi in xt_pre or pi >= total_pairs:
                    return
                pp = pi % n_pairs
                tiles = []
                for cc in range(6):
                    t = xtp.tile([128, F2], bf16, tag=f"xt{cc}", name=f"xt{pi}{cc}")
                    nc.sync.dma_start(
                        t[:], xt_d[pp, cc * 128:(cc + 1) * 128, :])
                    tiles.append(t)
                xt_pre[pi] = tiles

            # ---- constants (batched DMAs), ordered by first use: q/k
            # weights and the first x tile gate the very first matmul ----
            wt_dv = wt_d.rearrange("(cc p) j -> cc p j", cc=6).transpose((1, 0, 2))
            wtqk_all = constp.tile([128, 6 * 2 * C], bf16, tag="wtqk")
            nc.sync.dma_start(
                wtqk_all[:].rearrange("p (cc j) -> p cc j", cc=6),
                wt_dv[:, :, 0:2 * C])
            load_xt(0)
            wtv_all = constp.tile([128, 6 * C], bf16, tag="wtv")
            nc.sync.dma_start(
                wtv_all[:].rearrange("p (cc j) -> p cc j", cc=6),
                wt_dv[:, :, 2 * C:3 * C])
            wtqk_sb = [wtqk_all[:, cc * 2 * C:(cc + 1) * 2 * C] for cc in range(6)]
            wtv_sb = [wtv_all[:, cc * C:(cc + 1) * C] for cc in range(6)]
            pt_all = constp.tile([128, 6 * C], bf16, tag="ptall")
            nc.sync.dma_start(
                pt_all[:].rearrange("p (cc j) -> p cc j", cc=6),
                projt_d.rearrange("(cc p) j -> cc p j", cc=6)
                .transpose((1, 0, 2)))
            projt_sb = [pt_all[:, cc * C:(cc + 1) * C] for cc in range(6)]
            erpe_all = constp.tile([128, 12 * F2], bf16, tag="erpeall")
            nc.sync.dma_start(
                erpe_all[:].rearrange("p (g j) -> p g j", g=12),
                erpe_d.rearrange("h c p j -> (h c) p j").transpose((1, 0, 2)))
            erpe_sb = [(erpe_all[:, (2 * hp) * F2:(2 * hp + 1) * F2],
                        erpe_all[:, (2 * hp + 1) * F2:(2 * hp + 2) * F2])
                       for hp in range(6)]
            projb_bc = constp.tile([128, C], f32, tag="pbbc")
            nc.gpsimd.dma_start(
                projb_bc[:],
                projb_d[:].unsqueeze(1).broadcast_to((1, 128, C)))
            bq_sb = constp.tile([128, 6], f32, tag="bq")
            nc.sync.dma_start(bq_sb[:], bq_d[:])
            cos_sb = constp.tile([128, F4], bf16, tag="cos")
            nc.sync.dma_start(cos_sb[:], cs_d[0])
            spm_sb = constp.tile([128, F4], bf16, tag="spm")
            nc.sync.dma_start(spm_sb[:], cs_d[1])
            # E-band: column 11 is ones; slicing [:, 11-h:23-h] gives a
            # [128, 12] selector with ones in column h. Columns 12-23 are
            # all-zero; [0:1, 12:24] serves as a zero lhsT for PSUM init.
            eband = constp.tile([128, 24], bf16, tag="eband")
            nc.vector.memset(eband[:], 0.0)
            nc.vector.memset(eband[:, 11:12], 1.0)

            qk_quad = {}
            v_pairs = {}
            att_state = {}

            def attention_phase1(p, par, rope_unit=None):
                """Scores/probs/attn@v/sums for image pair p (quad slot par).

                rope_unit(k), when given, emits the rope for m=k, k+6; units
                are interleaved into the head loop so the DVE FIFO serves
                attention's probs multiplies between rope units instead of
                after all of them.
                """
                v_sb = v_pairs.pop(p)
                ao_list = []
                sums_ps = sumsp.tile([12, F2], f32, tag="sums",
                                     padded_shape=[12, 512], name=f"sums{p}")
                # zero the sums bank (sets has_written) so the per-unit sums
                # matmuls can accumulate in any interleaving with start=False
                nc.tensor.matmul(
                    sums_ps[:], lhsT=eband[0:1, 12:24],
                    rhs=cos_sb[0:1, 0:F2], start=True, stop=False,
                    skip_group_check=True)
                if rope_unit is not None:
                    rope_unit(0)
                for hp in range(6):
                    qro = qk_quad[hp + 100]
                    kro = qk_quad[hp + 6 + 100]
                    ao = aops.tile([128, F2], f32, tag="aops",
                                   padded_shape=[128, 512], name=f"ao{p}{hp}")
                    for ic in range(2):
                        qoff = (par * 2 + ic) * N
                        # per-bank PE-tile rule: all start=True openers in
                        # one PSUM bank must share the lhsT base partition,
                        # so ph=0 groups live in ps0 and ph=1 groups in ps1
                        # (cols 0 and 256, both 1KB-aligned)
                        ps0 = scps.tile([128, 512], f32, tag="ps0",
                                        padded_shape=[128, 512], name=f"ps0{p}{hp}{ic}")
                        ps1 = scps.tile([128, 512], f32, tag="ps1",
                                        padded_shape=[128, 512], name=f"ps1{p}{hp}{ic}")
                        qv0 = qro[0:64, qoff:qoff + N]
                        nc.tensor.matmul(
                            ps0[:, 0:N], lhsT=kro[0:64, qoff:qoff + 128],
                            rhs=qv0, start=True, stop=True)
                        nc.tensor.matmul(
                            ps0[0:N1, 256:256 + N],
                            lhsT=kro[0:64, qoff + 128:qoff + N],
                            rhs=qv0, start=True, stop=True)
                        qv1 = qro[64:128, qoff:qoff + N]
                        nc.tensor.matmul(
                            ps1[:, 0:N], lhsT=kro[64:128, qoff:qoff + 128],
                            rhs=qv1, start=True, stop=True)
                        nc.tensor.matmul(
                            ps1[0:N1, 256:256 + N],
                            lhsT=kro[64:128, qoff + 128:qoff + N],
                            rhs=qv1, start=True, stop=True)
                        pr = probsp.tile([128, 2 * F2], bf16, tag="pr",
                                         name=f"pr{p}{hp}{ic}")
                        prA = pr[:, 0:F2]
                        prB = pr[0:128, F2:2 * F2]
                        # one exp per score bank: strided AP covers the A
                        # chunk and the B chunk (B rows >= N1 read stale
                        # psum, land in unread probs rows)
                        for ph, bank in ((0, ps0), (1, ps1)):
                            nc.scalar.activation(
                                pr[:].rearrange("q (c x i) -> q c x i",
                                                c=2, x=2)[:, :, ph, :],
                                bank[:].rearrange("q (c z) -> q c z",
                                                  c=2)[:, :, 0:N],
                                AF.Exp)
                        # keep the DVE fed: next rope unit goes ahead of the
                        # probs multiplies it does not depend on
                        if rope_unit is not None and ic == 0 and hp < 5:
                            rope_unit(hp + 1)
                        # rel-pos bias: probs *= exp(rpb); prA on Pool (off
                        # the rope-saturated DVE FIFO), small prB on DVE
                        nc.gpsimd.tensor_mul(prA, prA, erpe_sb[hp][0])
                        nc.vector.tensor_mul(prB[0:N1, :], prB[0:N1, :],
                                             erpe_sb[hp][1][0:N1, :])
                        for ph in range(2):
                            h = 2 * hp + ph
                            cr = ph * N
                            nc.tensor.matmul(
                                ao[ph * 64:(ph + 1) * 64, ic * N:(ic + 1) * N],
                                lhsT=v_sb[ic][0][:, h * 64:(h + 1) * 64],
                                rhs=prA[:, cr:cr + N], start=True, stop=False)
                            nc.tensor.matmul(
                                ao[ph * 64:(ph + 1) * 64, ic * N:(ic + 1) * N],
                                lhsT=v_sb[ic][1][0:N1, h * 64:(h + 1) * 64],
                                rhs=prB[0:N1, cr:cr + N], start=False, stop=True)
                            last = (hp == 5 and ic == 1 and ph == 1)
                            nc.tensor.matmul(
                                sums_ps[:, ic * N:(ic + 1) * N],
                                lhsT=eband[:, 11 - h:23 - h],
                                rhs=prA[:, cr:cr + N],
                                start=False, stop=False, skip_group_check=True)
                            nc.tensor.matmul(
                                sums_ps[:, ic * N:(ic + 1) * N],
                                lhsT=eband[0:N1, 11 - h:23 - h],
                                rhs=prB[0:N1, cr:cr + N],
                                start=False, stop=last, skip_group_check=True)
                    aot = aosbp.tile([128, F2], bf16, tag="aosb",
                                     name=f"aot{p}{hp}")
                    nc.scalar.activation(aot[:], ao[:], AF.Copy)
                    ao_list.append(aot)
                att_state[p] = (ao_list, sums_ps)

            y_flat = y_d.rearrange("img n c -> (img n) c")

            def attention_phase2a(p, half, otq):
                """Normalize (1/sums broadcast multiply) into the quad-wide
                O^T tiles; output projection happens at quad level."""
                ao_list, sums_ps = att_state.pop(p)
                # ---- normalization: r = 1/sums via fast DVE reciprocal ----
                rsf = rsbp.tile([12, F2], f32, tag="rsf", name=f"rsf{p}")
                nc.vector.reciprocal_approx_fast(rsf[:], sums_ps[:])
                # broadcast r rows across partitions via a DRAM bounce:
                # heads 2hp -> rows 0-63, heads 2hp+1 -> rows 64-127
                slot = p % 2
                # store half-major: dram[half, hp*F2 + i] = rsf[2*hp + half, i]
                nc.gpsimd.dma_start(
                    rsf_d[slot].rearrange("h (g i) -> g h i", g=6), rsf[:])
                rball = rbc.tile([128, 6 * F2], f32, tag="rbc", name=f"rb{p}")
                for hb in range(2):
                    nc.gpsimd.dma_start(
                        rball[hb * 64:(hb + 1) * 64, :],
                        rsf_d[slot, hb:hb + 1].unsqueeze(1)
                        .broadcast_to((1, 64, 6 * F2)))
                for hp in range(6):
                    nc.vector.tensor_mul(
                        otq[hp][:, half * F2:(half + 1) * F2],
                        ao_list[hp][:], rball[:, hp * F2:(hp + 1) * F2])

            def yproj_chunks(otq, row_base, chunks):
                """Output projection over quad-token chunks (can span image
                boundaries: the store writes a flattened [img*n, c] view)."""
                for t0, n_sz in chunks:
                    yt = ysbp.tile([128, C], f32, tag="ysb",
                                   name=f"yt{row_base}{t0}")
                    for ch in range(2):
                        ps = yps.tile([128, 384], f32, tag="yps",
                                      padded_shape=[128, 512],
                                      name=f"yps{row_base}{t0}{ch}")
                        for cc in range(6):
                            nc.tensor.matmul(
                                ps[0:n_sz, :],
                                lhsT=otq[cc][:, t0:t0 + n_sz],
                                rhs=projt_sb[cc][:, ch * 384:(ch + 1) * 384],
                                start=(cc == 0), stop=(cc == 5))
                        nc.vector.tensor_add(
                            yt[0:n_sz, ch * 384:(ch + 1) * 384],
                            ps[0:n_sz, :],
                            projb_bc[0:n_sz, ch * 384:(ch + 1) * 384])
                    nc.scalar.dma_start(
                        y_flat[row_base + t0:row_base + t0 + n_sz, :],
                        yt[0:n_sz, :])

            for pi in range(total_pairs):
                p = pi % n_pairs
                par = pi % 2
                load_xt(pi)
                xt_sb = xt_pre.pop(pi)

                # ---- q/k projection into quad tiles ----
                if par == 0:
                    for m in range(12):
                        qk_quad[m] = qkbfp.tile(
                            [128, F4], bf16, tag=f"qk{m}", name=f"qk{pi}{m}")
                for m in range(12):
                    ps = qkps.tile([128, F2], f32, tag="qkps",
                                   padded_shape=[128, 512], name=f"qkp{pi}{m}")
                    for cc in range(6):
                        nc.tensor.matmul(
                            ps[:],
                            lhsT=wtqk_sb[cc][:, m * 128:(m + 1) * 128],
                            rhs=xt_sb[cc][:],
                            start=(cc == 0), stop=(cc == 5))
                    dst = qk_quad[m][:, par * F2:(par + 1) * F2]
                    if m < 6:
                        # q eviction adds the (scaled, permuted) q bias so
                        # rope needs no scalar term
                        nc.scalar.activation(dst, ps[:], AF.Identity,
                                             bias=bq_sb[:, m:m + 1])
                    else:
                        nc.vector.tensor_copy(dst, ps[:])

                # ---- v projection (natural out) ----
                v_sb = []
                for ic in range(2):
                    vts = [vsbp.tile([128, C], bf16, tag="vsb",
                                     name=f"vsb{pi}{ic}{i}") for i in range(2)]
                    for nck, (n_off, n_sz) in enumerate(((0, N0), (N0, N1))):
                        for ch in range(2):
                            ps = yps.tile([128, 384], f32, tag="yps",
                                          padded_shape=[128, 512],
                                          name=f"vps{pi}{ic}{nck}{ch}")
                            for cc in range(6):
                                nc.tensor.matmul(
                                    ps[0:n_sz, :],
                                    lhsT=xt_sb[cc][:, ic * N + n_off:ic * N + n_off + n_sz],
                                    rhs=wtv_sb[cc][:, ch * 384:(ch + 1) * 384],
                                    start=(cc == 0), stop=(cc == 5))
                            nc.scalar.activation(
                                vts[nck][0:n_sz, ch * 384:(ch + 1) * 384],
                                ps[0:n_sz, :], AF.Copy)
                    v_sb.append(vts)
                v_pairs[p] = v_sb
                load_xt(pi + 1)  # prefetch next pair while attention runs

                # ---- rope + attention, software-pipelined over the quad ----
                if par == 1 or pi == total_pairs - 1:
                    fw = F4 if par == 1 else F2

                    def rope_unit(k, pi=pi, fw=fw):
                        """Rope m=k (q) and m=k+6 (k) of the current quad."""
                        for m in (k, k + 6):
                            src = qk_quad[m]
                            qs = ropet.tile([128, F4], bf16, tag="qs",
                                            name=f"qs{pi}{m}")
                            nc.vector.stream_shuffle(qs[:, 0:fw], src[:, 0:fw],
                                                     SHUF_MASK)
                            u = ropet.tile([128, F4], bf16, tag="u",
                                           name=f"u{pi}{m}")
                            v = ropet.tile([128, F4], bf16, tag="v",
                                           name=f"v{pi}{m}")
                            # bias already folded at eviction; all-bf16
                            # TensorTensor ops run in the 2x DVE mode
                            nc.vector.tensor_mul(u[:, 0:fw], src[:, 0:fw],
                                                 cos_sb[:, 0:fw])
                            nc.vector.tensor_mul(v[:, 0:fw], qs[:, 0:fw],
                                                 spm_sb[:, 0:fw])
                            nc.vector.tensor_add(src[:, 0:fw], u[:, 0:fw],
                                                 v[:, 0:fw])
                            qk_quad[m + 100] = src

                    prev = (pi - 1) % n_pairs
                    otq = [otp.tile([128, 2 * F2], bf16, tag="ot",
                                    name=f"otq{pi}{cc}") for cc in range(6)]
                    if par == 1:
                        attention_phase1(prev, 0, rope_unit)
                        attention_phase1(p, par)
                        attention_phase2a(prev, 0, otq)
                        # pairA-only token chunks run while pairB normalizes
                        yproj_chunks(otq, prev * 2 * N,
                                     [(0, 128), (128, 128), (256, 128)])
                        attention_phase2a(p, 1, otq)
                        yproj_chunks(otq, prev * 2 * N,
                                     [(384, 128), (512, 128), (640, 128),
                                      (768, 20)])
                    else:
                        attention_phase1(p, par, rope_unit)
                        attention_phase2a(p, 0, otq)
                        yproj_chunks(otq, p * 2 * N,
                                     [(0, 128), (128, 128), (256, 128),
                                      (384, 10)])
    nc.compile()
    return nc


def host_prepare(inputs):
    x = np.asarray(inputs["x"], np.float32)
    qkv_w = np.asarray(inputs["qkv_w"], np.float32)
    scale = D ** -0.5
    Wq = qkv_w[:C] + np.asarray(inputs["lora_q_b"]) @ np.asarray(inputs["lora_q_a"])
    Wk = qkv_w[C:2 * C] + np.asarray(inputs["lora_k_b"]) @ np.asarray(inputs["lora_k_a"])
    Wv = qkv_w[2 * C:] + np.asarray(inputs["lora_v_b"]) @ np.asarray(inputs["lora_v_a"])
    p64 = _perm64()
    perm = (np.arange(H)[:, None] * D + p64[None, :]).ravel()
    Wq_de = (Wq * scale)[perm]
    bq_de = (np.asarray(inputs["q_bias"], np.float32) * scale)[perm]
    Wk_de = Wk[perm]
    wt = np.ascontiguousarray(
        np.concatenate([Wq_de, Wk_de, Wv], 0).T).astype(ml_dtypes.bfloat16)

    bq = np.ascontiguousarray(bq_de.reshape(6, 128).T)

    cos_f = np.ones((N, D), np.float32)
    cos_f[1:] = np.asarray(inputs["rope_cos"], np.float32)
    sin_f = np.zeros((N, D), np.float32)
    sin_f[1:] = np.asarray(inputs["rope_sin"], np.float32)
    cos_de = np.ascontiguousarray(cos_f[:, p64].T)
    spm = np.ascontiguousarray(sin_f[:, p64].T)
    for blk in range(2):
        spm[blk * 32:blk * 32 + 16] *= -1.0
    cs = np.stack([
        np.tile(np.vstack([cos_de, cos_de]), (1, 4)),
        np.tile(np.vstack([spm, spm]), (1, 4)),
    ]).astype(ml_dtypes.bfloat16)

    rel_table = np.asarray(inputs["rel_table"], np.float32)
    rel_index = np.asarray(inputs["rel_index"])
    rpb = rel_table[rel_index.reshape(-1)].reshape(N, N, H)
    rpbT = rpb.transpose(2, 1, 0)  # [h, j, i]
    # rel-pos bias as a probs multiplier exp(rpb) for all heads
    erpe = np.ones((6, 2, 128, F2), np.float32)
    for hp in range(6):
        for ph in range(2):
            h = 2 * hp + ph
            erpe[hp, 0, :, ph * N:(ph + 1) * N] = np.exp(rpbT[h, 0:128, :])
            erpe[hp, 1, 0:N1, ph * N:(ph + 1) * N] = np.exp(rpbT[h, 128:N, :])
    erpe = erpe.astype(ml_dtypes.bfloat16)

    proj_w = np.asarray(inputs["proj_w"], np.float32)
    projt = np.ascontiguousarray(proj_w.T).astype(ml_dtypes.bfloat16)
    projb = (np.asarray(inputs["proj_b"], np.float32)
             + proj_w @ np.asarray(inputs["v_bias"], np.float32)).reshape(1, C)

    xt = x.transpose(0, 2, 1)  # [B, C, N]
    xt_pairs = np.ascontiguousarray(
        xt.reshape(B // 2, 2, C, N).transpose(0, 2, 1, 3)
        .reshape(B // 2, C, 2 * N)).astype(ml_dtypes.bfloat16)

    shared = dict(wt=wt, bq=bq, cs=cs, erpe=erpe,
                  projt=projt, projb=projb)
    per_core = []
    ppc = BPC // 2
    for c in range(NCORES):
        m = dict(shared)
        m["xt"] = np.ascontiguousarray(xt_pairs[c * ppc:(c + 1) * ppc])
        per_core.append(m)
    return per_core


def kernel(**inputs):
    from concourse.bass_utils import run_bass_kernel_spmd
    in_maps = host_prepare(inputs)
    if "nc" not in _cache:
        _cache["nc"] = build_program()
    nc = _cache["nc"]
    res = run_bass_kernel_spmd(nc, in_maps, list(range(NCORES))).results
    y = np.concatenate([res[c]["y"] for c in range(NCORES)], 0)
    return np.ascontiguousarray(y.astype(np.float32))


# revision 48
# speedup vs baseline: 1.3129x; 1.3129x over previous
"""EVA-02 ViT attention block (LoRA + rope + rel-pos-bias) on 8 TRN2 NeuronCores.

Data-parallel over batch (8 images per core), all matmuls bf16 (1 cyc/row).
Per core:
  - LoRA merged into qkv weights on the host; q-scale and v-bias folded away.
  - q/k projected in transposed layout (channels on partitions), v natural.
  - q-bias added during PSUM eviction (Act Identity+bias); rope is pure
    bf16 tensor ops on DVE (2x mode), pair-swap via stream_shuffle.
  - scores transposed (S^T[j,i]), one matmul per (head, j-chunk). Walrus
    quadrant-tile constraint: all start=True openers within one PSUM bank
    must share the lhsT base partition, so ph=0 heads (base 0) and ph=1
    heads (base 64) accumulate in separate banks, at 1KB-aligned offsets.
  - rel-pos bias folded multiplicatively: probs *= exp(rpb) (host table),
    A-chunk on Pool, B-chunk on DVE — off the rope-saturated DVE FIFO.
  - exp on ScalarE without max subtraction (scores are O(1)); probs bf16,
    both j-chunks of a head-pair written by one strided-AP activation.
  - attn@v with v stationary emits O^T; softmax denominators via
    ones-vector matmuls (first one opens the PSUM group: start granularity
    is the whole bank); 1/x via DVE reciprocal_approx_fast (avoids the
    Ln/Exp act-table ping-pong); reciprocals broadcast to all partitions
    through a DRAM bounce (SBUF->SBUF DMA cannot replicate rows).
  - per-quad software pipeline: rope units interleaved into the first
    attention's head loop; phase1 (scores..sums) of both pairs runs before
    normalization so PE never waits on the reciprocal chain; the output
    projection runs on quad-wide O^T tiles (788 tokens -> 7 chunk-groups
    instead of 8, stores through a flattened [img*n, c] DRAM view);
    xt prefetched one pair ahead; first-use-ordered constant DMAs
    shorten the cold start.
"""
import numpy as np
import ml_dtypes

B, N, C, H, R = 64, 197, 768, 12, 24
D = C // H               # 64
NCORES = 8
BPC = B // NCORES        # images per core
F2 = 2 * N               # 394
F4 = 4 * N               # 788
N0, N1 = 128, N - 128    # token chunks: 128 + 69

_cache = {}

SHUF_MASK = list(range(16, 32)) + list(range(0, 16))
ROPE_ORDER = [0, 6, 1, 7, 2, 8, 3, 9, 4, 10, 5, 11]


def _perm64():
    p = []
    for blk in range(2):
        base = blk * 32
        p += [base + 2 * t for t in range(16)]
        p += [base + 2 * t + 1 for t in range(16)]
    return np.array(p)


def build_program(n_pairs=BPC // 2, use_shuffle=True, repeat=1):
    import concourse.bass as bass
    import concourse.tile as tile
    from concourse import bacc, mybir

    f32, f32r, bf16 = mybir.dt.float32, mybir.dt.float32r, mybir.dt.bfloat16
    AF = mybir.ActivationFunctionType
    OP = mybir.AluOpType

    nc = bacc.Bacc("TRN2", target_bir_lowering=False, debug=False)
    n_img = 2 * n_pairs

    xt_d = nc.dram_tensor("xt", [n_pairs, C, F2], bf16, kind="ExternalInput")
    wt_d = nc.dram_tensor("wt", [C, 3 * C], bf16, kind="ExternalInput")
    bq_d = nc.dram_tensor("bq", [128, 6], f32, kind="ExternalInput")
    cs_d = nc.dram_tensor("cs", [2, 128, F4], bf16, kind="ExternalInput")
    erpe_d = nc.dram_tensor("erpe", [6, 2, 128, F2], bf16, kind="ExternalInput")
    projt_d = nc.dram_tensor("projt", [C, C], bf16, kind="ExternalInput")
    projb_d = nc.dram_tensor("projb", [1, C], f32, kind="ExternalInput")
    y_d = nc.dram_tensor("y", [n_img, N, C], f32, kind="ExternalOutput")
    # DRAM bounce buffer for the softmax-reciprocal broadcast (SBUF->SBUF
    # DMAs cannot replicate one partition row to many)
    rsf_d = nc.dram_tensor("rsf_scratch", [2, 2, 6 * F2], f32, kind="Internal")

    from contextlib import ExitStack
    with tile.TileContext(nc) as tc:
        with ExitStack() as stk:
            pool = lambda name, bufs, **kw: stk.enter_context(
                tc.tile_pool(name=name, bufs=bufs, **kw))
            # NOTE: bufs is per-tag. PSUM budget: qkps 2 + yps 2 (shared by
            # v-proj and y-proj) + ps0 1 + ps1 1 + aops 1 + sums 1 = 8 banks.
            constp = pool("const", 1)
            xtp = pool("xt", 2)
            qkps = pool("qkps", 2, space="PSUM")
            qkbfp = pool("qkbf", 2)
            ropet = pool("ropet", 1)
            vsbp = pool("vsb", 8)
            scps = pool("scps", 1, space="PSUM")
            probsp = pool("probs", 4)
            aops = pool("aops", 1, space="PSUM")
            sumsp = pool("sums", 1, space="PSUM")
            rsbp = pool("rsb", 2)
            aosbp = pool("aosb", 12)
            rbc = pool("rbc", 1)
            yps = pool("yps", 2, space="PSUM")
            ysbp = pool("ysb", 2)
            otp = pool("otp", 6)

            total_pairs = repeat * n_pairs
            xt_pre = {}

            def load_xt(pi):
                if pi in xt_pre or pi >= total_pairs:
                    return
                pp = pi % n_pairs
                tiles = []
                for cc in range(6):
                    t = xtp.tile([128, F2], bf16, tag=f"xt{cc}", name=f"xt{pi}{cc}")
                    nc.sync.dma_start(
                        t[:], xt_d[pp, cc * 128:(cc + 1) * 128, :])
                    tiles.append(t)
                xt_pre[pi] = tiles

            # ---- constants (batched DMAs), ordered by first use: q/k
            # weights and the first x tile gate the very first matmul ----
            wt_dv = wt_d.rearrange("(cc p) j -> cc p j", cc=6).transpose((1, 0, 2))
            wtqk_all = constp.tile([128, 6 * 2 * C], bf16, tag="wtqk")
            nc.sync.dma_start(
                wtqk_all[:].rearrange("p (cc j) -> p cc j", cc=6),
                wt_dv[:, :, 0:2 * C])
            load_xt(0)
            wtv_all = constp.tile([128, 6 * C], bf16, tag="wtv")
            nc.sync.dma_start(
                wtv_all[:].rearrange("p (cc j) -> p cc j", cc=6),
                wt_dv[:, :, 2 * C:3 * C])
            wtqk_sb = [wtqk_all[:, cc * 2 * C:(cc + 1) * 2 * C] for cc in range(6)]
            wtv_sb = [wtv_all[:, cc * C:(cc + 1) * C] for cc in range(6)]
            pt_all = constp.tile([128, 6 * C], bf16, tag="ptall")
            nc.sync.dma_start(
                pt_all[:].rearrange("p (cc j) -> p cc j", cc=6),
                projt_d.rearrange("(cc p) j -> cc p j", cc=6)
                .transpose((1, 0, 2)))
            projt_sb = [pt_all[:, cc * C:(cc + 1) * C] for cc in range(6)]
            erpe_all = constp.tile([128, 12 * F2], bf16, tag="erpeall")
            nc.sync.dma_start(
                erpe_all[:].rearrange("p (g j) -> p g j", g=12),
                erpe_d.rearrange("h c p j -> (h c) p j").transpose((1, 0, 2)))
            erpe_sb = [(erpe_all[:, (2 * hp) * F2:(2 * hp + 1) * F2],
                        erpe_all[:, (2 * hp + 1) * F2:(2 * hp + 2) * F2])
                       for hp in range(6)]
            projb_bc = constp.tile([128, C], f32, tag="pbbc")
            nc.gpsimd.dma_start(
                projb_bc[:],
                projb_d[:].unsqueeze(1).broadcast_to((1, 128, C)))
            bq_sb = constp.tile([128, 6], f32, tag="bq")
            nc.sync.dma_start(bq_sb[:], bq_d[:])
            cos_sb = constp.tile([128, F4], bf16, tag="cos")
            nc.sync.dma_start(cos_sb[:], cs_d[0])
            spm_sb = constp.tile([128, F4], bf16, tag="spm")
            nc.sync.dma_start(spm_sb[:], cs_d[1])
            # E-band: column 11 is ones; slicing [:, 11-h:23-h] gives a
            # [128, 12] selector with ones in column h. Columns 12-23 are
            # all-zero; [0:1, 12:24] serves as a zero lhsT for PSUM init.
            eband = constp.tile([128, 24], bf16, tag="eband")
            nc.vector.memset(eband[:], 0.0)
            nc.vector.memset(eband[:, 11:12], 1.0)

            qk_quad = {}
            v_pairs = {}
            att_state = {}

            def attention_phase1(p, par, rope_unit=None):
                """Scores/probs/attn@v/sums for image pair p (quad slot par).

                rope_unit(k), when given, emits the rope for m=k, k+6; units
                are interleaved into the head loop so the DVE FIFO serves
                attention's probs multiplies between rope units instead of
                after all of them.
                """
                v_sb = v_pairs.pop(p)
                ao_list = []
                sums_ps = sumsp.tile([12, F2], f32, tag="sums",
                                     padded_shape=[12, 512], name=f"sums{p}")
                # the first sums matmul (hp=0, ic=0, ph=0, A-chunk) opens the
                # accumulation group; PSUM start granularity is the whole
                # 2KB bank, so later start=False writes to other columns are
                # covered
                if rope_unit is not None:
                    rope_unit(0)
                for hp in range(6):
                    qro = qk_quad[hp + 100]
                    kro = qk_quad[hp + 6 + 100]
                    ao = aops.tile([128, F2], f32, tag="aops",
                                   padded_shape=[128, 512], name=f"ao{p}{hp}")
                    for ic in range(2):
                        qoff = (par * 2 + ic) * N
                        # per-bank PE-tile rule: all start=True openers in
                        # one PSUM bank must share the lhsT base partition,
                        # so ph=0 groups live in ps0 and ph=1 groups in ps1
                        # (cols 0 and 256, both 1KB-aligned)
                        ps0 = scps.tile([128, 512], f32, tag="ps0",
                                        padded_shape=[128, 512], name=f"ps0{p}{hp}{ic}")
                        ps1 = scps.tile([128, 512], f32, tag="ps1",
                                        padded_shape=[128, 512], name=f"ps1{p}{hp}{ic}")
                        qv0 = qro[0:64, qoff:qoff + N]
                        nc.tensor.matmul(
                            ps0[:, 0:N], lhsT=kro[0:64, qoff:qoff + 128],
                            rhs=qv0, start=True, stop=True)
                        nc.tensor.matmul(
                            ps0[0:N1, 256:256 + N],
                            lhsT=kro[0:64, qoff + 128:qoff + N],
                            rhs=qv0, start=True, stop=True)
                        qv1 = qro[64:128, qoff:qoff + N]
                        nc.tensor.matmul(
                            ps1[:, 0:N], lhsT=kro[64:128, qoff:qoff + 128],
                            rhs=qv1, start=True, stop=True)
                        nc.tensor.matmul(
                            ps1[0:N1, 256:256 + N],
                            lhsT=kro[64:128, qoff + 128:qoff + N],
                            rhs=qv1, start=True, stop=True)
                        pr = probsp.tile([128, 2 * F2], bf16, tag="pr",
                                         name=f"pr{p}{hp}{ic}")
                        prA = pr[:, 0:F2]
                        prB = pr[0:128, F2:2 * F2]
                        # one exp per score bank: strided AP covers the A
                        # chunk and the B chunk (B rows >= N1 read stale
                        # psum, land in unread probs rows)
                        for ph, bank in ((0, ps0), (1, ps1)):
                            nc.scalar.activation(
                                pr[:].rearrange("q (c x i) -> q c x i",
                                                c=2, x=2)[:, :, ph, :],
                                bank[:].rearrange("q (c z) -> q c z",
                                                  c=2)[:, :, 0:N],
                                AF.Exp)
                        # keep the DVE fed: next rope unit goes ahead of the
                        # probs multiplies it does not depend on
                        if rope_unit is not None and ic == 0 and hp < 5:
                            rope_unit(hp + 1)
                        # rel-pos bias: probs *= exp(rpb); prA on Pool (off
                        # the rope-saturated DVE FIFO), small prB on DVE
                        nc.gpsimd.tensor_mul(prA, prA, erpe_sb[hp][0])
                        nc.vector.tensor_mul(prB[0:N1, :], prB[0:N1, :],
                                             erpe_sb[hp][1][0:N1, :])
                        for ph in range(2):
                            h = 2 * hp + ph
                            cr = ph * N
                            nc.tensor.matmul(
                                ao[ph * 64:(ph + 1) * 64, ic * N:(ic + 1) * N],
                                lhsT=v_sb[ic][0][:, h * 64:(h + 1) * 64],
                                rhs=prA[:, cr:cr + N], start=True, stop=False)
                            nc.tensor.matmul(
                                ao[ph * 64:(ph + 1) * 64, ic * N:(ic + 1) * N],
                                lhsT=v_sb[ic][1][0:N1, h * 64:(h + 1) * 64],
                                rhs=prB[0:N1, cr:cr + N], start=False, stop=True)
                            last = (hp == 5 and ic == 1 and ph == 1)
                            first = (hp == 0 and ic == 0 and ph == 0)
                            nc.tensor.matmul(
                                sums_ps[:, ic * N:(ic + 1) * N],
                                lhsT=eband[:, 11 - h:23 - h],
                                rhs=prA[:, cr:cr + N],
                                start=first, stop=False, skip_group_check=True)
                            nc.tensor.matmul(
                                sums_ps[:, ic * N:(ic + 1) * N],
                                lhsT=eband[0:N1, 11 - h:23 - h],
                                rhs=prB[0:N1, cr:cr + N],
                                start=False, stop=last, skip_group_check=True)
                    aot = aosbp.tile([128, F2], bf16, tag="aosb",
                                     name=f"aot{p}{hp}")
                    nc.scalar.activation(aot[:], ao[:], AF.Copy)
                    ao_list.append(aot)
                att_state[p] = (ao_list, sums_ps)

            y_flat = y_d.rearrange("img n c -> (img n) c")

            def attention_phase2a(p, half, otq):
                """Normalize (1/sums broadcast multiply) into the quad-wide
                O^T tiles; output projection happens at quad level."""
                ao_list, sums_ps = att_state.pop(p)
                # ---- normalization: r = 1/sums via fast DVE reciprocal ----
                rsf = rsbp.tile([12, F2], f32, tag="rsf", name=f"rsf{p}")
                nc.vector.reciprocal_approx_fast(rsf[:], sums_ps[:])
                # broadcast r rows across partitions via a DRAM bounce:
                # heads 2hp -> rows 0-63, heads 2hp+1 -> rows 64-127
                slot = p % 2
                # store half-major: dram[half, hp*F2 + i] = rsf[2*hp + half, i]
                nc.gpsimd.dma_start(
                    rsf_d[slot].rearrange("h (g i) -> g h i", g=6), rsf[:])
                rball = rbc.tile([128, 6 * F2], f32, tag="rbc", name=f"rb{p}")
                for hb in range(2):
                    nc.gpsimd.dma_start(
                        rball[hb * 64:(hb + 1) * 64, :],
                        rsf_d[slot, hb:hb + 1].unsqueeze(1)
                        .broadcast_to((1, 64, 6 * F2)))
                for hp in range(6):
                    nc.vector.tensor_mul(
                        otq[hp][:, half * F2:(half + 1) * F2],
                        ao_list[hp][:], rball[:, hp * F2:(hp + 1) * F2])

            def yproj_chunks(otq, row_base, chunks):
                """Output projection over quad-token chunks (can span image
                boundaries: the store writes a flattened [img*n, c] view)."""
                for t0, n_sz in chunks:
                    yt = ysbp.tile([128, C], f32, tag="ysb",
                                   name=f"yt{row_base}{t0}")
                    for ch in range(2):
                        ps = yps.tile([128, 384], f32, tag="yps",
                                      padded_shape=[128, 512],
                                      name=f"yps{row_base}{t0}{ch}")
                        for cc in range(6):
                            nc.tensor.matmul(
                                ps[0:n_sz, :],
                                lhsT=otq[cc][:, t0:t0 + n_sz],
                                rhs=projt_sb[cc][:, ch * 384:(ch + 1) * 384],
                                start=(cc == 0), stop=(cc == 5))
                        nc.vector.tensor_add(
                            yt[0:n_sz, ch * 384:(ch + 1) * 384],
                            ps[0:n_sz, :],
                            projb_bc[0:n_sz, ch * 384:(ch + 1) * 384])
                    nc.scalar.dma_start(
                        y_flat[row_base + t0:row_base + t0 + n_sz, :],
                        yt[0:n_sz, :])

            for pi in range(total_pairs):
                p = pi % n_pairs
                par = pi % 2
                load_xt(pi)
                xt_sb = xt_pre.pop(pi)

                # ---- q/k projection into quad tiles ----
                if par == 0:
                    for m in range(12):
                        qk_quad[m] = qkbfp.tile(
                            [128, F4], bf16, tag=f"qk{m}", name=f"qk{pi}{m}")
                for m in range(12):
                    ps = qkps.tile([128, F2], f32, tag="qkps",
                                   padded_shape=[128, 512], name=f"qkp{pi}{m}")
                    for cc in range(6):
                        nc.tensor.matmul(
                            ps[:],
                            lhsT=wtqk_sb[cc][:, m * 128:(m + 1) * 128],
                            rhs=xt_sb[cc][:],
                            start=(cc == 0), stop=(cc == 5))
                    dst = qk_quad[m][:, par * F2:(par + 1) * F2]
                    if m < 6:
                        # q eviction adds the (scaled, permuted) q bias so
                        # rope needs no scalar term
                        nc.scalar.activation(dst, ps[:], AF.Identity,
                                             bias=bq_sb[:, m:m + 1])
                    else:
                        nc.vector.tensor_copy(dst, ps[:])

                # ---- v projection (natural out) ----
                v_sb = []
                for ic in range(2):
                    vts = [vsbp.tile([128, C], bf16, tag="vsb",
                                     name=f"vsb{pi}{ic}{i}") for i in range(2)]
                    for nck, (n_off, n_sz) in enumerate(((0, N0), (N0, N1))):
                        for ch in range(2):
                            ps = yps.tile([128, 384], f32, tag="yps",
                                          padded_shape=[128, 512],
                                          name=f"vps{pi}{ic}{nck}{ch}")
                            for cc in range(6):
                                nc.tensor.matmul(
                                    ps[0:n_sz, :],
                                    lhsT=xt_sb[cc][:, ic * N + n_off:ic * N + n_off + n_sz],
                                    rhs=wtv_sb[cc][:, ch * 384:(ch + 1) * 384],
                                    start=(cc == 0), stop=(cc == 5))
                            nc.scalar.activation(
                                vts[nck][0:n_sz, ch * 384:(ch + 1) * 384],
                                ps[0:n_sz, :], AF.Copy)
                    v_sb.append(vts)
                v_pairs[p] = v_sb
                load_xt(pi + 1)  # prefetch next pair while attention runs

                # ---- rope + attention, software-pipelined over the quad ----
                if par == 1 or pi == total_pairs - 1:
                    fw = F4 if par == 1 else F2

                    def rope_unit(k, pi=pi, fw=fw):
                        """Rope m=k (q) and m=k+6 (k) of the current quad."""
                        for m in (k, k + 6):
                            src = qk_quad[m]
                            qs = ropet.tile([128, F4], bf16, tag="qs",
                                            name=f"qs{pi}{m}")
                            nc.vector.stream_shuffle(qs[:, 0:fw], src[:, 0:fw],
                                                     SHUF_MASK)
                            u = ropet.tile([128, F4], bf16, tag="u",
                                           name=f"u{pi}{m}")
                            v = ropet.tile([128, F4], bf16, tag="v",
                                           name=f"v{pi}{m}")
                            # bias already folded at eviction; all-bf16
                            # TensorTensor ops run in the 2x DVE mode
                            nc.vector.tensor_mul(u[:, 0:fw], src[:, 0:fw],
                                                 cos_sb[:, 0:fw])
                            nc.vector.tensor_mul(v[:, 0:fw], qs[:, 0:fw],
                                                 spm_sb[:, 0:fw])
                            nc.vector.tensor_add(src[:, 0:fw], u[:, 0:fw],
                                                 v[:, 0:fw])
                            qk_quad[m + 100] = src

                    prev = (pi - 1) % n_pairs
                    otq = [otp.tile([128, 2 * F2], bf16, tag="ot",
                                    name=f"otq{pi}{cc}") for cc in range(6)]
                    if par == 1:
                        attention_phase1(prev, 0, rope_unit)
                        attention_phase1(p, par)
                        attention_phase2a(prev, 0, otq)
                        # pairA-only token chunks run while pairB normalizes
                        yproj_chunks(otq, prev * 2 * N,
                                     [(0, 128), (128, 128), (256, 128)])
                        attention_phase2a(p, 1, otq)
                        yproj_chunks(otq, prev * 2 * N,
                                     [(384, 128), (512, 128), (640, 128),
                                      (768, 20)])
                    else:
                        attention_phase1(p, par, rope_unit)
                        attention_phase2a(p, 0, otq)
                        yproj_chunks(otq, p * 2 * N,
                                     [(0, 128), (128, 128), (256, 128),
                                      (384, 10)])
    nc.compile()
    return nc


def host_prepare(inputs):
    x = np.asarray(inputs["x"], np.float32)
    qkv_w = np.asarray(inputs["qkv_w"], np.float32)
    scale = D ** -0.5
    Wq = qkv_w[:C] + np.asarray(inputs["lora_q_b"]) @ np.asarray(inputs["lora_q_a"])
    Wk = qkv_w[C:2 * C] + np.asarray(inputs["lora_k_b"]) @ np.asarray(inputs["lora_k_a"])
    Wv = qkv_w[2 * C:] + np.asarray(inputs["lora_v_b"]) @ np.asarray(inputs["lora_v_a"])
    p64 = _perm64()
    perm = (np.arange(H)[:, None] * D + p64[None, :]).ravel()
    Wq_de = (Wq * scale)[perm]
    bq_de = (np.asarray(inputs["q_bias"], np.float32) * scale)[perm]
    Wk_de = Wk[perm]
    wt = np.ascontiguousarray(
        np.concatenate([Wq_de, Wk_de, Wv], 0).T).astype(ml_dtypes.bfloat16)

    bq = np.ascontiguousarray(bq_de.reshape(6, 128).T)

    cos_f = np.ones((N, D), np.float32)
    cos_f[1:] = np.asarray(inputs["rope_cos"], np.float32)
    sin_f = np.zeros((N, D), np.float32)
    sin_f[1:] = np.asarray(inputs["rope_sin"], np.float32)
    cos_de = np.ascontiguousarray(cos_f[:, p64].T)
    spm = np.ascontiguousarray(sin_f[:, p64].T)
    for blk in range(2):
        spm[blk * 32:blk * 32 + 16] *= -1.0
    cs = np.stack([
        np.tile(np.vstack([cos_de, cos_de]), (1, 4)),
        np.tile(np.vstack([spm, spm]), (1, 4)),
    ]).astype(ml_dtypes.bfloat16)

    rel_table = np.asarray(inputs["rel_table"], np.float32)
    rel_index = np.asarray(inputs["rel_index"])
    rpb = rel_table[rel_index.reshape(-1)].reshape(N, N, H)
    rpbT = rpb.transpose(2, 1, 0)  # [h, j, i]
    # rel-pos bias as a probs multiplier exp(rpb) for all heads
    erpe = np.ones((6, 2, 128, F2), np.float32)
    for hp in range(6):
        for ph in range(2):
            h = 2 * hp + ph
            erpe[hp, 0, :, ph * N:(ph + 1) * N] = np.exp(rpbT[h, 0:128, :])
            erpe[hp, 1, 0:N1, ph * N:(ph + 1) * N] = np.exp(rpbT[h, 128:N, :])
    erpe = erpe.astype(ml_dtypes.bfloat16)

    proj_w = np.asarray(inputs["proj_w"], np.float32)
    projt = np.ascontiguousarray(proj_w.T).astype(ml_dtypes.bfloat16)
    projb = (np.asarray(inputs["proj_b"], np.float32)
             + proj_w @ np.asarray(inputs["v_bias"], np.float32)).reshape(1, C)

    xt = x.transpose(0, 2, 1)  # [B, C, N]
    xt_pairs = np.ascontiguousarray(
        xt.reshape(B // 2, 2, C, N).transpose(0, 2, 1, 3)
        .reshape(B // 2, C, 2 * N)).astype(ml_dtypes.bfloat16)

    shared = dict(wt=wt, bq=bq, cs=cs, erpe=erpe,
                  projt=projt, projb=projb)
    per_core = []
    ppc = BPC // 2
    for c in range(NCORES):
        m = dict(shared)
        m["xt"] = np.ascontiguousarray(xt_pairs[c * ppc:(c + 1) * ppc])
        per_core.append(m)
    return per_core


def kernel(**inputs):
    from concourse.bass_utils import run_bass_kernel_spmd
    in_maps = host_prepare(inputs)
    if "nc" not in _cache:
        _cache["nc"] = build_program()
    nc = _cache["nc"]
    res = run_bass_kernel_spmd(nc, in_maps, list(range(NCORES))).results
    y = np.concatenate([res[c]["y"] for c in range(NCORES)], 0)
    return np.ascontiguousarray(y.astype(np.float32))
